# revision 5
# baseline (speedup 1.0000x reference)
import sys

if "/opt/trn_rl_repo" not in sys.path:
    sys.path.insert(0, "/opt/trn_rl_repo")

from contextlib import ExitStack

import numpy as np
import concourse.bass as bass
import concourse.mybir as mybir
from concourse.bass_utils import run_bass_kernel_spmd

# Problem: loss = sum_b ||cos(2pi(output_b-0.5))|| * ||cos(2pi(target_b-0.5))||
# for output/target of shape [4096, 4096] f32, values in [0, 1).
#
# Math used on device: with theta = 2pi*x - pi (in [-pi, pi), where the Sin
# LUT is accurate), s = sin(theta) and cos^2(2pi*(x-0.5)) = 1 - s^2. So per-row
# sumsq = N - sum(s^2). The device returns per-row-segment partial sum(s^2);
# sqrt/product/final sum happen on host in float64.
#
# Layout: each core's [512, 4096] shard is viewed as [128, 16384] so partition
# p holds rows 4p..4p+3 as one 64 KiB contiguous DRAM line. Column-range DMA
# chunks then move 16-32 KiB contiguous per-partition lines (vs 8 KiB for
# row-block tiles), which is the main lever on HBM stream rate. The whole
# 16.8 MB shard is SBUF-resident (inputs f32 + sin bf16 = ~197 KiB/partition),
# so no buffer-reuse semaphore waits exist anywhere.

B, N = 4096, 4096
N_CORES = 8
ROWS_PER_CORE = B // N_CORES  # 512
P = 128
COLS = ROWS_PER_CORE * N // P  # 16384 per tensor per partition
TWO_PI = 2.0 * np.pi

# Per-tensor DVE reduce segments: 4096-col groups (one per row within the
# partition); group 3 is split to taper the pipeline tail.
_SEGMENTS = [
    (0, 4096),
    (4096, 8192),
    (8192, 12288),
    (12288, 14336),
    (14336, 15360),
    (15360, 16384),
]
N_ACC = 2 * len(_SEGMENTS)  # 12 acc columns: output segs then target segs

# DMA chunks (col ranges of the [128, 16384] view), front-loaded: biggest
# first so the last-landing bytes carry the least trailing ACT work.
_CHUNK_SPANS = [(0, 8192), (8192, 12288), (12288, 14336), (14336, 15360), (15360, 16384)]
# Interleave output/target: (tensor_idx, c0, c1)
_CHUNKS = []
for span in _CHUNK_SPANS:
    _CHUNKS.append((0, span[0], span[1]))
    _CHUNKS.append((1, span[0], span[1]))
N_CHUNKS = len(_CHUNKS)  # 10

# Map: for each chunk, which segments it fully contains (seg indices per tensor)
_SEG_OF_CHUNK = []
for t_idx, c0, c1 in _CHUNKS:
    segs = [i for i, (s0, s1) in enumerate(_SEGMENTS) if s0 >= c0 and s1 <= c1]
    _SEG_OF_CHUNK.append(segs)

# acc column of (tensor, seg) in DVE issue order
_ACC_COL = {}
_col = 0
for ci, (t_idx, c0, c1) in enumerate(_CHUNKS):
    for si in _SEG_OF_CHUNK[ci]:
        _ACC_COL[(t_idx, si)] = _col
        _col += 1
assert _col == N_ACC

# DVE instr count after which acc[:, :SPLIT] is complete (all cols written by
# the first out-DMA must have been produced). Cols are assigned in DVE order,
# so after k DVE instrs, cols 0..k-1 are done.
_OUT_SPLIT = N_ACC - 2  # last two DVE instrs (o-g3c, t-g3c) go in out-DMA 2

_CACHE = {}


def _build():
    nc = bass.Bass()
    o_ext = nc.declare_dram_parameter(
        "output", [P, COLS], mybir.dt.float32, isOutput=False
    )
    t_ext = nc.declare_dram_parameter(
        "target", [P, COLS], mybir.dt.float32, isOutput=False
    )
    acc_ext = nc.declare_dram_parameter(
        "acc", [P, N_ACC], mybir.dt.float32, isOutput=True
    )
    exts = (o_ext, t_ext)

    one_ap = nc.const_aps.tensor(1.0, (P, 1), mybir.dt.float32)

    with (
        ExitStack() as ctx,
        nc.semaphore("dma_sem") as dma_sem,
        nc.semaphore("act_sem") as act_sem,
        nc.semaphore("dve_sem") as dve_sem,
        nc.Block(no_gpsimd_drain=True) as block,
    ):
        in_bufs = [
            ctx.enter_context(
                nc.sbuf_tensor(f"in_{n}", [P, COLS], mybir.dt.float32)
            )
            for n in ("o", "t")
        ]
        # bf16 sin values: halves DVE bytes (2x perf mode); the f32 accumulator
        # keeps the sum accurate (sum error ~1e-4 rel).
        res_bufs = [
            ctx.enter_context(
                nc.sbuf_tensor(f"res_{n}", [P, COLS], mybir.dt.bfloat16)
            )
            for n in ("o", "t")
        ]
        scratch = ctx.enter_context(
            nc.sbuf_tensor("scratch", [P, 1], mybir.dt.bfloat16)
        )
        bias_t = ctx.enter_context(
            nc.sbuf_tensor("bias_neg_pi", [P, 1], mybir.dt.float32)
        )
        acc = ctx.enter_context(
            nc.sbuf_tensor("acc_sb", [P, N_ACC], mybir.dt.float32)
        )

        @block.sync
        def _(sync):
            # Whole shard is resident: issue every input DMA immediately.
            for t_idx, c0, c1 in _CHUNKS:
                sync.dma_start(
                    out=in_bufs[t_idx][:, c0:c1], in_=exts[t_idx][:, c0:c1]
                ).then_inc(dma_sem, 16)
            # Early out-DMA for the bulk of acc; tail-gated one for the rest.
            sync.wait_ge(dve_sem, _OUT_SPLIT)
            sync.dma_start(
                out=acc_ext[:, :_OUT_SPLIT], in_=acc[:, :_OUT_SPLIT]
            ).then_inc(dma_sem, 16)
            sync.wait_ge(dve_sem, N_ACC)
            # Sem update is mandatory for HWDGE codegen, but nothing waits on
            # it: the block-end InstDrain on SP retires this DMA before the
            # NEFF completes.
            sync.dma_start(
                out=acc_ext[:, _OUT_SPLIT:], in_=acc[:, _OUT_SPLIT:]
            ).then_inc(dma_sem, 16)

        @block.scalar
        def _(scalar):
            # bias_t = -pi, produced on the consuming engine (no cross-engine
            # sync needed; the pre-registered const-1.0 AP is barrier-ready).
            scalar.mul(bias_t[:], one_ap, float(-np.pi))
            for ci, (t_idx, c0, c1) in enumerate(_CHUNKS):
                scalar.wait_ge(dma_sem, 16 * (ci + 1))
                scalar.activation(
                    res_bufs[t_idx][:, c0:c1],
                    in_bufs[t_idx][:, c0:c1],
                    mybir.ActivationFunctionType.Sin,
                    bias=bias_t[:],
                    scale=TWO_PI,
                ).then_inc(act_sem, 1)

        @block.vector
        def _(vector):
            for ci, (t_idx, _, _) in enumerate(_CHUNKS):
                for si in _SEG_OF_CHUNK[ci]:
                    s0, s1 = _SEGMENTS[si]
                    w = s1 - s0
                    vector.wait_ge(act_sem, ci + 1)
                    vector.scalar_tensor_tensor(
                        out=scratch[:].broadcast_to([P, w]),
                        in0=res_bufs[t_idx][:, s0:s1],
                        scalar=1.0,
                        in1=res_bufs[t_idx][:, s0:s1],
                        op0=mybir.AluOpType.mult,
                        op1=mybir.AluOpType.mult,
                        accum_out=acc[:, _ACC_COL[(t_idx, si)] : _ACC_COL[(t_idx, si)] + 1],
                    ).then_inc(dve_sem, 1)

    return nc


def _get_nc():
    if "nc" not in _CACHE:
        _CACHE["nc"] = _build()
    return _CACHE["nc"]


def make_in_maps(output: np.ndarray, target: np.ndarray):
    return [
        {
            "output": output[c * ROWS_PER_CORE : (c + 1) * ROWS_PER_CORE].reshape(
                P, COLS
            ),
            "target": target[c * ROWS_PER_CORE : (c + 1) * ROWS_PER_CORE].reshape(
                P, COLS
            ),
        }
        for c in range(N_CORES)
    ]


def kernel(output: np.ndarray, target: np.ndarray) -> np.ndarray:
    output = np.ascontiguousarray(output, dtype=np.float32)
    target = np.ascontiguousarray(target, dtype=np.float32)
    nc = _get_nc()
    in_maps = make_in_maps(output, target)
    results = run_bass_kernel_spmd(nc, in_maps, core_ids=list(range(N_CORES))).results

    n_seg = len(_SEGMENTS)
    total = 0.0
    for c in range(N_CORES):
        acc = results[c]["acc"].astype(np.float64)  # [P, N_ACC]
        # Rebuild per-(tensor, group) sum of sin^2. Groups 0..2 are segments
        # 0..2; group 3 = segments 3+4+5. Row r of the shard = 4p + g.
        sumsq = np.zeros((2, P, 4), dtype=np.float64)
        for (t_idx, si), col in _ACC_COL.items():
            g = min(si, 3)
            sumsq[t_idx, :, g] += acc[:, col]
        so = np.maximum(float(N) - sumsq[0], 0.0)  # [P, 4] -> row 4p+g
        st = np.maximum(float(N) - sumsq[1], 0.0)
        total += np.sqrt(so * st).sum()
    return np.array(total, dtype=np.float32)


# revision 6
# speedup vs baseline: 1.0438x; 1.0438x over previous
import sys

if "/opt/trn_rl_repo" not in sys.path:
    sys.path.insert(0, "/opt/trn_rl_repo")

from contextlib import ExitStack

import numpy as np
import concourse.bass as bass
import concourse.mybir as mybir
from concourse.bass_utils import run_bass_kernel_spmd

# Problem: loss = sum_b ||cos(2pi(output_b-0.5))|| * ||cos(2pi(target_b-0.5))||
# for output/target of shape [4096, 4096] f32, values in [0, 1).
#
# Math used on device: with theta = 2pi*x - pi (in [-pi, pi), where the Sin
# LUT is accurate -- it extrapolates garbage beyond ~1.2pi, measured), the
# identity cos^2(2pi*(x-0.5)) = 1 - sin^2(theta) gives per-row sumsq =
# N - sum(sin^2). The device returns per-row-segment partial sum(sin^2);
# sqrt/product/final sum happen on host in float64.
#
# Layout: each core's [512, 4096] shard is viewed as [128, 16384] so partition
# p holds rows 4p..4p+3 as one 64 KiB contiguous DRAM line. Column-range DMA
# chunks move 2-16 KiB contiguous per-partition lines, which streams at
# ~410 GB/s vs ~340 GB/s for the old 8 KiB row-block tiling with buffer reuse.
# The whole 16.8 MB shard is SBUF-resident (inputs f32 + sin bf16 =
# ~197 KiB/partition), so no buffer-reuse semaphore waits exist anywhere.
#
# DVE square+reduce runs in 2x perf mode: all APs must be 2-byte and stride-1
# packed, so the elementwise product is written in-place over the (dead) sin
# values rather than to a 0-stride broadcast scratch (which forces 1x mode).

B, N = 4096, 4096
N_CORES = 8
ROWS_PER_CORE = B // N_CORES  # 512
P = 128
COLS = ROWS_PER_CORE * N // P  # 16384 per tensor per partition
TWO_PI = 2.0 * np.pi

# Chunk = DMA unit = ACT unit = DVE reduce segment (col range of the
# [128, 16384] view). 4096-col groups (one row per partition) with the last
# group split to taper the pipeline tail.
_SPANS = [
    (0, 4096),
    (4096, 8192),
    (8192, 12288),
    (12288, 14336),
    (14336, 15360),
    (15360, 16384),
]
# Interleave output/target: (tensor_idx, c0, c1); acc col = chunk index.
_CHUNKS = []
for span in _SPANS:
    _CHUNKS.append((0, span[0], span[1]))
    _CHUNKS.append((1, span[0], span[1]))
N_CHUNKS = len(_CHUNKS)  # 12
N_ACC = N_CHUNKS
_OUT_SPLIT = N_ACC - 2  # last two acc cols ride the tail-gated out-DMA

_CACHE = {}


def _build():
    nc = bass.Bass()
    o_ext = nc.declare_dram_parameter(
        "output", [P, COLS], mybir.dt.float32, isOutput=False
    )
    t_ext = nc.declare_dram_parameter(
        "target", [P, COLS], mybir.dt.float32, isOutput=False
    )
    acc_ext = nc.declare_dram_parameter(
        "acc", [P, N_ACC], mybir.dt.float32, isOutput=True
    )
    exts = (o_ext, t_ext)

    one_ap = nc.const_aps.tensor(1.0, (P, 1), mybir.dt.float32)

    with (
        ExitStack() as ctx,
        nc.semaphore("dma_sem") as dma_sem,
        nc.semaphore("act_sem") as act_sem,
        nc.semaphore("dve_sem") as dve_sem,
        nc.Block(no_gpsimd_drain=True) as block,
    ):
        in_bufs = [
            ctx.enter_context(
                nc.sbuf_tensor(f"in_{n}", [P, COLS], mybir.dt.float32)
            )
            for n in ("o", "t")
        ]
        # bf16 sin values: halves DVE bytes (2x perf mode); the f32 accumulator
        # keeps the sum accurate (sum error ~1e-4 rel).
        res_bufs = [
            ctx.enter_context(
                nc.sbuf_tensor(f"res_{n}", [P, COLS], mybir.dt.bfloat16)
            )
            for n in ("o", "t")
        ]
        bias_t = ctx.enter_context(
            nc.sbuf_tensor("bias_neg_pi", [P, 1], mybir.dt.float32)
        )
        acc = ctx.enter_context(
            nc.sbuf_tensor("acc_sb", [P, N_ACC], mybir.dt.float32)
        )

        @block.sync
        def _(sync):
            # Whole shard is resident: issue every input DMA immediately.
            for t_idx, c0, c1 in _CHUNKS:
                sync.dma_start(
                    out=in_bufs[t_idx][:, c0:c1], in_=exts[t_idx][:, c0:c1]
                ).then_inc(dma_sem, 16)
            # Early out-DMA for the bulk of acc; tail-gated one for the rest.
            sync.wait_ge(dve_sem, _OUT_SPLIT)
            sync.dma_start(
                out=acc_ext[:, :_OUT_SPLIT], in_=acc[:, :_OUT_SPLIT]
            ).then_inc(dma_sem, 16)
            sync.wait_ge(dve_sem, N_ACC)
            # Sem update is mandatory for HWDGE codegen, but nothing waits on
            # it: the block-end InstDrain on SP retires this DMA before the
            # NEFF completes.
            sync.dma_start(
                out=acc_ext[:, _OUT_SPLIT:], in_=acc[:, _OUT_SPLIT:]
            ).then_inc(dma_sem, 16)

        @block.scalar
        def _(scalar):
            # bias_t = -pi, produced on the consuming engine (no cross-engine
            # sync needed; the pre-registered const-1.0 AP is barrier-ready).
            scalar.mul(bias_t[:], one_ap, float(-np.pi))
            for ci, (t_idx, c0, c1) in enumerate(_CHUNKS):
                scalar.wait_ge(dma_sem, 16 * (ci + 1))
                scalar.activation(
                    res_bufs[t_idx][:, c0:c1],
                    in_bufs[t_idx][:, c0:c1],
                    mybir.ActivationFunctionType.Sin,
                    bias=bias_t[:],
                    scale=TWO_PI,
                ).then_inc(act_sem, 1)

        @block.vector
        def _(vector):
            for ci, (t_idx, c0, c1) in enumerate(_CHUNKS):
                w = c1 - c0
                vector.wait_ge(act_sem, ci + 1)
                vector.scalar_tensor_tensor(
                    out=res_bufs[t_idx][:, c0:c1],
                    in0=res_bufs[t_idx][:, c0:c1],
                    scalar=1.0,
                    in1=res_bufs[t_idx][:, c0:c1],
                    op0=mybir.AluOpType.mult,
                    op1=mybir.AluOpType.mult,
                    accum_out=acc[:, ci : ci + 1],
                ).then_inc(dve_sem, 1)

    return nc


def _get_nc():
    if "nc" not in _CACHE:
        _CACHE["nc"] = _build()
    return _CACHE["nc"]


def make_in_maps(output: np.ndarray, target: np.ndarray):
    return [
        {
            "output": output[c * ROWS_PER_CORE : (c + 1) * ROWS_PER_CORE].reshape(
                P, COLS
            ),
            "target": target[c * ROWS_PER_CORE : (c + 1) * ROWS_PER_CORE].reshape(
                P, COLS
            ),
        }
        for c in range(N_CORES)
    ]


def kernel(output: np.ndarray, target: np.ndarray) -> np.ndarray:
    output = np.ascontiguousarray(output, dtype=np.float32)
    target = np.ascontiguousarray(target, dtype=np.float32)
    nc = _get_nc()
    in_maps = make_in_maps(output, target)
    results = run_bass_kernel_spmd(nc, in_maps, core_ids=list(range(N_CORES))).results

    total = 0.0
    for c in range(N_CORES):
        acc = results[c]["acc"].astype(np.float64)  # [P, N_ACC]
        # Per-(tensor, group) sum of sin^2. Chunks alternate o/t per span;
        # spans 0..2 are groups 0..2, spans 3..5 all fold into group 3.
        # Row r of the shard = 4p + g.
        sumsq = np.zeros((2, P, 4), dtype=np.float64)
        for ci, (t_idx, c0, c1) in enumerate(_CHUNKS):
            g = min(c0 // 4096, 3)
            sumsq[t_idx, :, g] += acc[:, ci]
        so = np.maximum(float(N) - sumsq[0], 0.0)  # [P, 4] -> row 4p+g
        st = np.maximum(float(N) - sumsq[1], 0.0)
        total += np.sqrt(so * st).sum()
    return np.array(total, dtype=np.float32)
